# revision 1
# baseline (speedup 1.0000x reference)
"""Trainium2 Bass kernel for nn_DemoEnhancer (neural 3D-LUT image enhancer).

SPMD on 8 NeuronCores, image sharded by row blocks:
  A. Vertex MLP (fp16 matmuls, fp32 accum) on a 1/8 vertex shard per core.
  B. AllGather of the 33^3 LUT across cores.
  C. Build an r-expanded bilinear-coefficient table in DRAM:
     row index = (b*32+g)*513 + r16 with r16 = round(512*r);
     row = 32B: C0[rgb] f32 | C1..C3[rgb] bf16 | pad.
  D. Stream pixels: indices/fractions on DVE, one 32B indirect-DMA gather
     per pixel, bilinear combine in (g,b) on DVE.
"""
import sys, os

for _p in ("/opt/trn_rl_repo",):
    if _p not in sys.path and os.path.isdir(_p):
        sys.path.insert(0, _p)

import numpy as np

D = 33
NROWS = 32 * 32 * 32     # one row per cell, int16-indexable
ROWU = 128               # 16-bit units per table row (256B, dma_gather elem)
GCHUNK = 8192            # gather indices per dma_gather call
H, W = 2160, 3840
NCORES = 8
ROWS_PER_CORE = H // NCORES          # 270
NPIX = ROWS_PER_CORE * W             # 1036800
FREE = NPIX // 128                   # 8100 per partition
TILE_T = 768
TILES = [TILE_T] * (FREE // TILE_T) + ([FREE % TILE_T] if FREE % TILE_T else [])
HID = 256
NV = D ** 3                          # 35937
NV_PAD = 36864                       # 8 * 4608
VSHARD = NV_PAD // NCORES            # 4608
NSUB = 512                           # matmul N-subtile
KC = HID // 128                      # 2

_CACHE = {}


def _host_constants():
    v = np.arange(NV)
    r = (v % D) / (D - 1.0)
    g = ((v // D) % D) / (D - 1.0)
    b = (v // (D * D)) / (D - 1.0)
    coords = np.stack([r, g, b])
    srt = np.sort(coords, axis=0)
    xc = np.zeros((6, NV_PAD), np.float16)
    xc[:3, :NV] = coords
    xc[3:, :NV] = srt
    return (xc,)


def _build_program():
    import concourse.bass as bass
    import concourse.bacc as bacc
    import concourse.mybir as mybir
    import concourse.tile as tile

    fp32, fp16, bf16, i32 = (mybir.dt.float32, mybir.dt.float16,
                             mybir.dt.bfloat16, mybir.dt.int32)
    AF = mybir.ActivationFunctionType
    ALU = mybir.AluOpType

    nc = bacc.Bacc("TRN2", target_bir_lowering=False, debug=False,
                   num_devices=NCORES)

    img_d = nc.dram_tensor("img", [3, 128, FREE], fp32, kind="ExternalInput")
    param_d = nc.dram_tensor("param_t", [5, 1], fp32, kind="ExternalInput")
    w_d = {}
    for name, shape in [("W0", [11, HID]), ("W1", [HID, HID]),
                        ("W2", [HID, HID]), ("W3", [HID, HID]),
                        ("W4", [HID, 3])]:
        w_d[name] = nc.dram_tensor(name, shape, fp32, kind="ExternalInput")
    for name in ("b0", "b1", "b2", "b3"):
        w_d[name] = nc.dram_tensor(name, [HID], fp32, kind="ExternalInput")
    xc_d = nc.dram_tensor("xc", [6, VSHARD], fp16, kind="ExternalInput")

    shard_d = nc.dram_tensor("shard_i", [VSHARD * 3], fp32)
    lfull_d = nc.dram_tensor("lfull_i", [NV_PAD * 3], fp32, addr_space="Shared")
    table_d = nc.dram_tensor("table_i", [NROWS, ROWU], bf16)
    vols_d = nc.dram_tensor("vols_i", [4, D, 32, 32, 3], fp32)
    out_d = nc.dram_tensor("out", [3, 128, FREE], fp32, kind="ExternalOutput")

    with tile.TileContext(nc) as tc:
        # ============ Phase A: vertex MLP on this core's shard ============
        with tc.tile_pool(name="wpool", bufs=1) as wp, \
             tc.tile_pool(name="mlp", bufs=1) as mp, \
             tc.tile_pool(name="psmm", bufs=4, space="PSUM") as ps_mm, \
             tc.tile_pool(name="pssm", bufs=2, space="PSUM") as ps_sm:

            def load_w16(name, rows):
                """Load rows [kc*128:(kc+1)*128] of weight as fp16 tiles."""
                outts = []
                for kc in range(rows // 128):
                    t32 = wp.tile([128, w_d[name].shape[1]], fp32, tag=f"{name}32")
                    nc.sync.dma_start(t32[:], w_d[name].ap()[kc * 128:(kc + 1) * 128, :])
                    t16 = wp.tile([128, w_d[name].shape[1]], fp16, tag=f"{name}16_{kc}")
                    nc.vector.tensor_copy(t16[:], t32[:])
                    outts.append(t16)
                return outts

            w0_32 = wp.tile([6, HID], fp32)
            nc.sync.dma_start(w0_32[:3, :], w_d["W0"].ap()[0:3, :])
            nc.sync.dma_start(w0_32[3:6, :], w_d["W0"].ap()[8:11, :])
            w0cs = wp.tile([6, HID], fp16)
            nc.vector.tensor_copy(w0cs[:], w0_32[:])
            w0p = wp.tile([5, HID], fp32)
            nc.sync.dma_start(w0p[:], w_d["W0"].ap()[3:8, :])
            w1 = load_w16("W1", HID)
            w2 = load_w16("W2", HID)
            w3 = load_w16("W3", HID)
            w4 = load_w16("W4", HID)
            w4n = [wp.tile([128, 3], fp16, tag=f"w4n{kc}", name=f"w4n{kc}") for kc in range(KC)]
            for kc in range(KC):
                nc.vector.tensor_scalar_mul(w4n[kc][:], w4[kc][:], -1.0)

            par = wp.tile([5, 1], fp32)
            nc.sync.dma_start(par[:], param_d.ap()[:])
            biases = {}
            for bn in ("b0", "b1", "b2", "b3"):
                bt = wp.tile([128, KC], fp32, tag=bn)
                nc.sync.dma_start(bt[:], w_d[bn].ap().rearrange("(c p) -> p c", p=128))
                biases[bn] = bt
            b0e = wp.tile([128, KC], fp32)
            for c in range(KC):
                pe = ps_sm.tile([128, 1], fp32, tag="b0e", space="PSUM")
                nc.tensor.matmul(pe[:], lhsT=w0p[:, c * 128:(c + 1) * 128],
                                 rhs=par[:], start=True, stop=True)
                nc.vector.tensor_add(b0e[:, c:c + 1], pe[:], biases["b0"][:, c:c + 1])

            xc_sb = mp.tile([6, VSHARD], fp16)
            nc.sync.dma_start(xc_sb[:], xc_d.ap()[:])

            h = {"e": [mp.tile([128, VSHARD], fp16, tag=f"hpe{c}0", name=f"hae{c}") for c in range(KC)],
                 "z": [mp.tile([128, VSHARD], fp16, tag=f"hpz{c}0", name=f"haz{c}") for c in range(KC)]}
            nsub = VSHARD // NSUB
            for c in range(KC):
                for s in range(nsub):
                    sl = slice(s * NSUB, (s + 1) * NSUB)
                    ps = ps_mm.tile([128, NSUB], fp32, tag="mm", space="PSUM")
                    nc.tensor.matmul(ps[:], lhsT=w0cs[:, c * 128:(c + 1) * 128],
                                     rhs=xc_sb[:, sl], start=True, stop=True)
                    nc.scalar.activation(h["e"][c][:, sl], ps[:], AF.Relu,
                                         bias=b0e[:, c:c + 1], scale=1.0)
                    nc.scalar.activation(h["z"][c][:, sl], ps[:], AF.Relu,
                                         bias=biases["b0"][:, c:c + 1], scale=1.0)
            for li, wmat in ((1, w1), (2, w2), (3, w3)):
                hn = {k: [mp.tile([128, VSHARD], fp16, tag=f"hp{k}{c}{li % 2}", name=f"h{k}{li}{c}")
                          for c in range(KC)] for k in ("e", "z")}
                for k in ("e", "z"):
                    for c in range(KC):
                        for s in range(nsub):
                            sl = slice(s * NSUB, (s + 1) * NSUB)
                            ps = ps_mm.tile([128, NSUB], fp32, tag="mm", space="PSUM")
                            for kc in range(KC):
                                nc.tensor.matmul(
                                    ps[:], lhsT=wmat[kc][:, c * 128:(c + 1) * 128],
                                    rhs=h[k][kc][:, sl],
                                    start=(kc == 0), stop=(kc == KC - 1))
                            nc.scalar.activation(hn[k][c][:, sl], ps[:], AF.Tanh,
                                                 bias=biases[f"b{li}"][:, c:c + 1],
                                                 scale=1.0)
                h = hn
            lout = mp.tile([3, VSHARD], fp32)
            for s in range(nsub):
                sl = slice(s * NSUB, (s + 1) * NSUB)
                ps = ps_sm.tile([3, NSUB], fp32, tag="mm3", space="PSUM")
                nc.tensor.matmul(ps[:], lhsT=w4[0][:], rhs=h["e"][0][:, sl],
                                 start=True, stop=False)
                nc.tensor.matmul(ps[:], lhsT=w4[1][:], rhs=h["e"][1][:, sl],
                                 start=False, stop=False)
                nc.tensor.matmul(ps[:], lhsT=w4n[0][:], rhs=h["z"][0][:, sl],
                                 start=False, stop=False)
                nc.tensor.matmul(ps[:], lhsT=w4n[1][:], rhs=h["z"][1][:, sl],
                                 start=False, stop=True)
                nc.vector.tensor_add(lout[:, sl], ps[:], xc_sb[:3, sl])
            nc.sync.dma_start(shard_d.ap().rearrange("(v c) -> c v", c=3), lout[:])

        # ============ Phase B: AllGather LUT ============
        nc.gpsimd.collective_compute(
            "AllGather", mybir.AluOpType.bypass,
            replica_groups=[list(range(NCORES))],
            ins=[shard_d.ap()[:]], outs=[lfull_d.ap()[:]])

        # ============ Phase C: per-cell coefficient table ============
        with tc.tile_pool(name="cvol", bufs=1) as cp:
            lr = cp.tile([D, D, D, 3], fp32)
            nc.sync.dma_start(
                lr[:],
                lfull_d.ap()[:NV * 3].rearrange("(b g r c) -> r b g c",
                                                b=D, g=D, r=D, c=3))
            s00 = lr[:, 0:32, 0:32, :]
            s01 = lr[:, 0:32, 1:33, :]
            s10 = lr[:, 1:33, 0:32, :]
            s11 = lr[:, 1:33, 1:33, :]
            c1v = cp.tile([D, 32, 32, 3], fp32)
            c2v = cp.tile([D, 32, 32, 3], fp32)
            c3v = cp.tile([D, 32, 32, 3], fp32)
            nc.vector.tensor_tensor(c1v[:], s01, s00, op=ALU.subtract)
            nc.vector.tensor_tensor(c2v[:], s10, s00, op=ALU.subtract)
            nc.vector.tensor_tensor(c3v[:], s11, s10, op=ALU.subtract)
            nc.vector.tensor_tensor(c3v[:], c3v[:], c1v[:], op=ALU.subtract)
            vap = vols_d.ap()
            nc.sync.dma_start(vap[0, :, :, :, :], s00)
            nc.sync.dma_start(vap[1, :, :, :, :], c1v[:])
            nc.sync.dma_start(vap[2, :, :, :, :], c2v[:])
            nc.sync.dma_start(vap[3, :, :, :, :], c3v[:])
            # assemble 256B rows: [C0r0(6u) C0r1(6u) C123r0(9u) C123r1(9u) pad]
            vflat = vols_d.ap().rearrange("a r b g c -> a (r b g c)")
            tb32 = table_d.ap().bitcast(fp32).rearrange("n u -> (n u)")
            tb16 = table_d.ap().rearrange("n u -> (n u)")
            for rs in (0, 1):
                for bq in range(4):
                    bo = bq * 8
                    sap = bass.AP(vflat.tensor, vflat.offset + rs * 3072 + bo * 96,
                                  [[96, 8], [3, 32], [3072, 32], [1, 3]])
                    dap = bass.AP(tb32.tensor, rs * 3 + bo * 64 * 1024,
                                  [[64 * 1024, 8], [64 * 32, 32], [64, 32], [1, 3]])
                    nc.sync.dma_start(dap, sap)
                    for a in (1, 2, 3):
                        sapa = bass.AP(vflat.tensor,
                                       vflat.offset + a * (33 * 3072) + rs * 3072 + bo * 96,
                                       [[96, 8], [3, 32], [3072, 32], [1, 3]])
                        dapa = bass.AP(tb16.tensor,
                                       12 + rs * 9 + (a - 1) * 3 + bo * 128 * 1024,
                                       [[128 * 1024, 8], [128 * 32, 32], [128, 32], [1, 3]])
                        nc.gpsimd.dma_start(dapa, sapa)

        # ============ Phase D: pixel streaming ============
        NBLK = GCHUNK // 128
        with tc.tile_pool(name="pix", bufs=2) as pp, \
             tc.tile_pool(name="gat", bufs=2) as gp:
            for ti, T in enumerate(TILES):
                toff = ti * TILE_T
                ncall = (T * 128 + GCHUNK - 1) // GCHUNK
                x = [pp.tile([128, TILE_T, 1], fp32, tag=f"x{c}", name=f"x{c}") for c in range(3)]
                for c in range(3):
                    nc.sync.dma_start(x[c][:, :T, :],
                                      img_d.ap()[c, :, toff:toff + T])
                ii = []
                ff = []
                for c in range(3):
                    t_i = pp.tile([128, TILE_T, 1], i32, tag="itmp", bufs=3, name=f"ti{c}")
                    nc.vector.tensor_scalar(t_i[:, :T, :], x[c][:, :T, :], 32.0, 0.5,
                                            ALU.mult, ALU.subtract)
                    t_f = pp.tile([128, TILE_T, 1], fp32, tag="ftmp", bufs=3, name=f"tf{c}")
                    nc.vector.tensor_copy(t_f[:, :T, :], t_i[:, :T, :])
                    ii.append(t_i)
                    ff.append(t_f)
                # fractions F = [fr, fg, fb] (bf16)
                F = pp.tile([128, TILE_T, 3], bf16, tag="F")
                for c in range(3):
                    nc.vector.scalar_tensor_tensor(F[:, :T, c:c + 1], x[c][:, :T, :],
                                                   32.0, ff[c][:, :T, :],
                                                   ALU.mult, ALU.subtract)
                m3 = pp.tile([128, TILE_T, 1], bf16, tag="m3")
                nc.vector.tensor_tensor(m3[:, :T, :], F[:, :T, 1:2], F[:, :T, 2:3],
                                        op=ALU.mult)
                # cell index (int16): (ib*32+ig)*32+ir
                s1 = pp.tile([128, TILE_T, 1], fp32, tag="s1")
                nc.vector.scalar_tensor_tensor(s1[:, :T, :], ff[2][:, :T, :], 32.0,
                                               ff[1][:, :T, :], ALU.mult, ALU.add)
                i16 = pp.tile([128, TILE_T, 1], mybir.dt.int16, tag="i16")
                nc.vector.scalar_tensor_tensor(i16[:, :T, :], s1[:, :T, :], 32.0,
                                               ff[0][:, :T, :], ALU.mult, ALU.add)
                # wrapped idx layout: stream pos j=(t*128+p) -> (p%16, t*8+p//16)
                wrap = pp.tile([128, TILE_T * 8], mybir.dt.int16, tag="wrap", bufs=1)
                for ph in range(8):
                    wap = bass.AP(wrap.tensor, wrap[:].offset + ph,
                                  [[wrap[:].ap[0][0], 16], [8, T]])
                    nc.sync.dma_start(wap, i16[:, :T, 0].rearrange("p t -> p t")[ph * 16:(ph + 1) * 16, :])
                for gi in range(1, 8):
                    nc.sync.dma_start(wrap[:].rearrange("p f -> p f")[gi * 16:(gi + 1) * 16, :T * 8],
                                      wrap[:].rearrange("p f -> p f")[0:16, :T * 8])
                # gather + combine per chunk
                Gc = pp.tile([128, TILE_T, 30], bf16, tag="Gc", bufs=1)
                tin = bass.AP(table_d.ap().tensor, 0, [[ROWU, NROWS], [1, ROWU]])
                for ci in range(ncall):
                    nblk = min(NBLK, T - ci * NBLK)
                    G = gp.tile([128, NBLK, ROWU], bf16, tag="G", name="Gt")
                    nc.gpsimd.dma_gather(
                        out_ap=G[:, :nblk, :], in_ap=tin,
                        idxs_ap=wrap[:, ci * NBLK * 8:(ci * NBLK + nblk) * 8],
                        num_idxs=nblk * 128, num_idxs_reg=nblk * 128,
                        elem_size=ROWU, single_packet=False)
                    nc.vector.tensor_copy(
                        Gc[:, ci * NBLK:ci * NBLK + nblk, :],
                        G[:, :nblk, 0:30])
                Gf = Gc[:].bitcast(fp32)       # [128, TILE_T, 15]
                # bilinear in (g,b) per r-half, then r-mix
                orr = []
                for rs in (0, 1):
                    Pr = pp.tile([128, TILE_T, 3], bf16, tag="P", bufs=4, name=f"P{rs}")
                    du = 12 + rs * 9
                    nc.vector.tensor_tensor(
                        Pr[:, :T, :], Gc[:, :T, du:du + 3],
                        F[:, :T, 1:2].to_broadcast([128, T, 3]), op=ALU.mult)
                    P2 = pp.tile([128, TILE_T, 3], bf16, tag="P", bufs=4, name=f"P2{rs}")
                    nc.vector.tensor_tensor(
                        P2[:, :T, :], Gc[:, :T, du + 3:du + 6],
                        F[:, :T, 2:3].to_broadcast([128, T, 3]), op=ALU.mult)
                    nc.vector.tensor_tensor(Pr[:, :T, :], Pr[:, :T, :], P2[:, :T, :],
                                            op=ALU.add)
                    P3 = pp.tile([128, TILE_T, 3], bf16, tag="P", bufs=4, name=f"P3{rs}")
                    nc.vector.tensor_tensor(
                        P3[:, :T, :], Gc[:, :T, du + 6:du + 9],
                        m3[:, :T, :].to_broadcast([128, T, 3]), op=ALU.mult)
                    nc.vector.tensor_tensor(Pr[:, :T, :], Pr[:, :T, :], P3[:, :T, :],
                                            op=ALU.add)
                    orx = pp.tile([128, TILE_T, 3], fp32, tag=f"or{rs}", bufs=1, name=f"or{rs}")
                    nc.vector.tensor_tensor(orx[:, :T, :], Gf[:, :T, rs * 3:rs * 3 + 3],
                                            Pr[:, :T, :], op=ALU.add)
                    orr.append(orx)
                dmix = pp.tile([128, TILE_T, 3], fp32, tag="dmix", bufs=1)
                nc.vector.tensor_tensor(dmix[:, :T, :], orr[1][:, :T, :],
                                        orr[0][:, :T, :], op=ALU.subtract)
                nc.vector.tensor_tensor(dmix[:, :T, :], dmix[:, :T, :],
                                        F[:, :T, 0:1].to_broadcast([128, T, 3]),
                                        op=ALU.mult)
                out3 = pp.tile([128, TILE_T, 3], fp32, tag="out3", bufs=1)
                nc.vector.tensor_tensor(out3[:, :T, :], orr[0][:, :T, :],
                                        dmix[:, :T, :], op=ALU.add)
                for c in range(3):
                    nc.sync.dma_start(out_d.ap()[c, :, toff:toff + T],
                                      out3[:, :T, c:c + 1])

    nc.compile()
    return nc


def _get_program():
    if "nc" not in _CACHE:
        _CACHE["nc"] = _build_program()
        _CACHE["consts"] = _host_constants()
    return _CACHE["nc"], _CACHE["consts"]


def kernel(img, param, W0, b0, W1, b1, W2, b2, W3, b3, W4, b4):
    from concourse.bass_utils import run_bass_kernel_spmd

    nc, (xc,) = _get_program()
    img = np.ascontiguousarray(np.asarray(img, dtype=np.float32))
    base = {}
    for name, v in (("W0", W0), ("W1", W1), ("W2", W2), ("W3", W3), ("W4", W4),
                    ("b0", b0), ("b1", b1), ("b2", b2), ("b3", b3)):
        base[name] = np.ascontiguousarray(np.asarray(v, np.float32))
    base["param_t"] = np.asarray(param, np.float32).reshape(1, 5).T.copy()
    in_maps = []
    for i in range(NCORES):
        m = dict(base)
        m["img"] = np.ascontiguousarray(
            img[0, :, i * ROWS_PER_CORE:(i + 1) * ROWS_PER_CORE, :]
            .reshape(3, 128, FREE))
        m["xc"] = np.ascontiguousarray(xc[:, i * VSHARD:(i + 1) * VSHARD])
        in_maps.append(m)
    res = run_bass_kernel_spmd(nc, in_maps, core_ids=list(range(NCORES)),
                               trace=os.environ.get("KERNEL_TRACE", "0") == "1")
    _CACHE["last_result"] = res
    out = np.empty((1, 3, H, W), np.float32)
    for i in range(NCORES):
        out[0, :, i * ROWS_PER_CORE:(i + 1) * ROWS_PER_CORE, :] = \
            res.results[i]["out"].reshape(3, ROWS_PER_CORE, W)
    return out

